# revision 18
# baseline (speedup 1.0000x reference)
"""Trainium2 Bass kernel for nn_CoherenceLoss (topk-masked coherence/diversity loss).

Strategy (8 NeuronCores, column-sharded per the sharding hint):
  - W [8192, 8192] is sharded column-wise: core c owns columns [1024c, 1024c+1024),
    split into two 512-wide groups so group-0's reduction tail overlaps group-1's
    matmul stream. W is host-permuted to a partition-major layout so every DMA
    moves fat contiguous lines; each tensor streams as ~2MB dma_starts (each
    dma_start is spread over all 16 SDMA engines by the hardware).
  - beta [100, 8192] is replicated; each core computes the top-20 threshold t20
    per row (hierarchical max8 on DVE), the masked unnormalized softmax p in
    TRANSPOSED layout (host supplies a permuted beta^T), and M = p @ W_slice on
    the PE in fp32r (full-rate fp32; raw fp32 bits are accepted bit-identically
    to DVE-rounded fp32r).
  - All row-normalizations are deferred: each core emits per-topic partials
    [min M, max M, sum e^2, sum e^2*M, sum e^2*Md, sum e^2*Md*M, rowsum e, t20]
    and the host combines 8x[100,16] -> final scalar (exact algebra, validated
    against the reference at ~5e-6 relative error).

Math notes:
  - mask = (beta >= t20) equals the top-20 index set (no ties in the data).
  - p need not be normalized: Wc = (mx-M)/(mx-mn) is invariant to per-row
    positive scaling of M, so p_un = exp(beta-4)*mask suffices.
  - softmax(beta)^2 = e^2/R^2 with e = exp(beta-4), R = rowsum(e); 1/R^2 is
    applied on host.
  - Md = (colsum(mask) > mask) elementwise; colsum is over the 100 topics and
    is local to each column slice.
"""

import os
import numpy as np
from contextlib import ExitStack

N_CORES = 8
K = 100          # topics
V = 8192         # vocab
CS = V // N_CORES            # 1024 columns per core
G = 512                      # column group width (2 groups per core)
KT = 64                      # contraction tiles of 128
NCH = 8                      # transposed-layout chunks
WCK = 16                     # k-tiles per W DMA chunk (4 MB each)
LAMBDA_D = 0.7
LAMBDA_A = 100.0
WARMUP_EPOCHS = 100          # int(0.5 * 200)
SHIFT = 4.0                  # exp shift (any constant ~rowmax)

# W matmul dtype mode: "fp32r_raw" (DMA raw fp32 bits as fp32r) | "fp32"
W_MODE = os.environ.get("COH_W_MODE", "fp32r_raw")

TRACE = False                # test harness sets True for profiling
LAST_RESULT = None

_COMPILED = None


def _build():
    import concourse.tile as tile
    from concourse import bacc, mybir

    f32 = mybir.dt.float32
    f32r = mybir.dt.float32r
    A = mybir.AluOpType
    ACT = mybir.ActivationFunctionType
    w_dt = f32r if W_MODE == "fp32r_raw" else f32

    nc = bacc.Bacc("TRN2", debug=False, enable_asserts=False, num_devices=N_CORES)

    beta_ap = nc.dram_tensor("beta", [K, V], f32, kind="ExternalInput").ap()
    # betaTp[p, kt*K + t] = beta[t, 128*kt + p]  (host-permuted)
    betaTp_ap = nc.dram_tensor("betaTp", [128, KT * K], f32,
                               kind="ExternalInput").ap()
    beta_s_ap = nc.dram_tensor("beta_s", [K, CS], f32, kind="ExternalInput").ap()
    # wp{g}[p, kt*G + n] = W[128*kt + p, 1024c + g*G + n]  (host-permuted)
    w_aps = [nc.dram_tensor(f"wp{g}", [128, KT * G], f32,
                            kind="ExternalInput").ap() for g in range(2)]
    ident_ap = nc.dram_tensor("ident", [K, K], f32, kind="ExternalInput").ap()
    out_ap = nc.dram_tensor("out16", [K, 16], f32, kind="ExternalOutput").ap()

    with tile.TileContext(nc) as tc:
        with ExitStack() as ctx:
            big = ctx.enter_context(tc.tile_pool(name="big", bufs=1))
            chpool = ctx.enter_context(tc.tile_pool(name="ch", bufs=2))
            epool = ctx.enter_context(tc.tile_pool(name="ep", bufs=2))
            wpool = ctx.enter_context(tc.tile_pool(name="w", bufs=2))
            small = ctx.enter_context(tc.tile_pool(name="small", bufs=1))
            tpool = ctx.enter_context(tc.tile_pool(name="tails", bufs=2))
            psum = ctx.enter_context(tc.tile_pool(name="psA", bufs=1, space="PSUM"))
            psm = ctx.enter_context(tc.tile_pool(name="psM", bufs=1, space="PSUM"))

            # ---- input DMAs (small/chunked first; W stream last) ----
            sb_betaT = big.tile([128, KT * K], f32)
            for ch in range(2):
                sl = slice(ch * (KT // 2) * K, (ch + 1) * (KT // 2) * K)
                nc.sync.dma_start(sb_betaT[:, sl], betaTp_ap[:, sl])
            sb_beta_s = small.tile([K, CS], f32)
            nc.sync.dma_start(sb_beta_s[:], beta_s_ap[:])
            ident = small.tile([K, K], f32)
            nc.sync.dma_start(ident[:], ident_ap[:])

            bias4_100 = small.tile([K, 1], f32)
            nc.vector.memset(bias4_100[:], -SHIFT)
            bias8_100 = small.tile([K, 1], f32)
            nc.vector.memset(bias8_100[:], -2.0 * SHIFT)
            bias4_128 = small.tile([128, 1], f32)
            nc.vector.memset(bias4_128[:], -SHIFT)
            ones100 = small.tile([K, 1], f32)
            nc.gpsimd.memset(ones100[:], 1.0)
            ones1 = small.tile([1, 128], f32)
            nc.gpsimd.memset(ones1[:], 1.0)

            out16 = small.tile([K, 16], f32)

            # ---- stream beta chunks: stage-1 max8 segments + R-exp ----
            cand = small.tile([K, 256], f32)
            racc = small.tile([K, NCH], f32)
            for ch in range(NCH):
                sl = slice(ch * (V // NCH), (ch + 1) * (V // NCH))
                bchunk = epool.tile([K, V // NCH], f32, tag="bchunk")
                nc.sync.dma_start(bchunk[:], beta_ap[:, sl])
                for s in range(4):
                    seg = 4 * ch + s
                    nc.vector.max(cand[:, 8 * seg:8 * seg + 8],
                                  bchunk[:, 256 * s:256 * s + 256])
                esc = epool.tile([K, V // NCH], f32, tag="esc")
                nc.scalar.activation(esc[:], bchunk[:], ACT.Exp,
                                     bias=bias4_100[:], scale=1.0,
                                     accum_out=racc[:, ch:ch + 1])
            nc.vector.tensor_reduce(out16[:, 12:13], racc[:],
                                    axis=mybir.AxisListType.X, op=A.add)

            # ---- top-20 threshold (stage 2) ----
            m8a = small.tile([K, 8], f32)
            nc.vector.max(m8a[:], cand[:])
            cand2 = small.tile([K, 256], f32)
            nc.vector.match_replace(out=cand2[:], in_to_replace=m8a[:],
                                    in_values=cand[:], imm_value=-3e38)
            m8b = small.tile([K, 8], f32)
            nc.vector.max(m8b[:], cand2[:])
            cand3 = small.tile([K, 256], f32)
            nc.vector.match_replace(out=cand3[:], in_to_replace=m8b[:],
                                    in_values=cand2[:], imm_value=-3e38)
            m8c = small.tile([K, 8], f32)
            nc.vector.max(m8c[:], cand3[:])
            t20 = m8c[:, 3:4]   # 20th largest per row

            # ---- t20 into transposed layout: t20rep [128, (KT/NCH)*K] ----
            w100 = (KT // NCH) * K            # chunk width (800)
            ps_row = psum.tile([1, K], f32, tag="psrow")
            nc.tensor.transpose(ps_row[:], t20, ident[:])
            t20row = small.tile([1, K], f32)
            nc.scalar.copy(t20row[:], ps_row[:])
            t20rep = small.tile([128, w100], f32)
            rep_half = t20row[:, None].to_broadcast([1, (KT // NCH) // 2, K])
            for h in range(2):
                ps_bc = psum.tile([128, w100 // 2], f32, name=f"psbc{h}",
                                  tag=f"psbc{h}")
                nc.tensor.matmul(ps_bc[:], ones1[:], rep_half,
                                 start=True, stop=True)
                nc.scalar.copy(t20rep[:, h * (w100 // 2):(h + 1) * (w100 // 2)],
                               ps_bc[:])

            # ---- transposed-layout masked softmax: pT (fp32r) ----
            pT = big.tile([128, KT * K], f32r)
            for ch in range(NCH):
                sl = slice(ch * w100, (ch + 1) * w100)
                eT = chpool.tile([128, w100], f32, tag="eT")
                nc.scalar.activation(eT[:], sb_betaT[:, sl], ACT.Exp,
                                     bias=bias4_128[:], scale=1.0)
                maskT = chpool.tile([128, w100], f32, tag="maskT")
                nc.vector.tensor_tensor(out=maskT[:], in0=sb_betaT[:, sl],
                                        in1=t20rep[:], op=A.is_ge)
                nc.vector.tensor_tensor(out=pT[:, sl], in0=eT[:], in1=maskT[:],
                                        op=A.mult)

            nc.vector.tensor_copy(out16[:, 13:14], t20)

            # ---- M-independent tail inputs (need only t20 + beta_s) ----
            es_l, ew_l = [], []
            for g in range(2):
                o = 6 * g
                ms = tpool.tile([K, G], f32, name=f"ms{g}", tag=f"ms{g}")
                nc.vector.tensor_scalar(ms[:], sb_beta_s[:, g * G:(g + 1) * G],
                                        t20, None, op0=A.is_ge)
                ps_cs = psum.tile([1, G], f32, name=f"pscs{g}", tag="pscs")
                nc.tensor.matmul(ps_cs[:], ones100[:], ms[:],
                                 start=True, stop=True)
                cs = tpool.tile([1, G], f32, name=f"cs{g}", tag="cs")
                nc.scalar.copy(cs[:], ps_cs[:])
                ps_csbc = psum.tile([K, G], f32, name=f"pscsbc{g}", tag="pscsbc")
                nc.tensor.matmul(ps_csbc[:], ones1[:, :K], cs[:],
                                 start=True, stop=True)
                wmd = tpool.tile([K, G], f32, name=f"wmd{g}", tag=f"wmd{g}")
                nc.vector.tensor_tensor(out=wmd[:], in0=ps_csbc[:], in1=ms[:],
                                        op=A.is_gt)
                es_g = tpool.tile([K, G], f32, name=f"es{g}", tag=f"es{g}")
                nc.scalar.activation(es_g[:], sb_beta_s[:, g * G:(g + 1) * G],
                                     ACT.Exp, bias=bias8_100[:], scale=2.0,
                                     accum_out=out16[:, o + 2:o + 3])
                ew_g = tpool.tile([K, G], f32, name=f"ew{g}", tag=f"ew{g}")
                nc.vector.scalar_tensor_tensor(
                    ew_g[:], in0=es_g[:], scalar=1.0, in1=wmd[:],
                    op0=A.mult, op1=A.mult,
                    accum_out=out16[:, o + 4:o + 5])
                es_l.append(es_g)
                ew_l.append(ew_g)

            # ---- main matmul: M[g] = p_un @ W[:, g] (fp32r, 64 k-tiles) ----
            ps_M = [psm.tile([K, G], f32, name=f"psM{g}", tag=f"psM{g}")
                    for g in range(2)]
            for g in range(2):
                for ck in range(KT // WCK):
                    wt = wpool.tile([128, WCK * G], w_dt, tag="wt")
                    wsrc = w_aps[g][:, ck * WCK * G:(ck + 1) * WCK * G]
                    if w_dt is f32r:
                        wsrc = wsrc.bitcast(f32r)
                    nc.sync.dma_start(wt[:], wsrc)
                    for l in range(WCK):
                        kt = ck * WCK + l
                        nc.tensor.matmul(ps_M[g][:],
                                         pT[:, kt * K:(kt + 1) * K],
                                         wt[:, l * G:(l + 1) * G],
                                         start=(kt == 0), stop=(kt == KT - 1))

            # ---- per-group tails (M-dependent only) ----
            for g in range(2):
                o = 6 * g
                Msb = tpool.tile([K, G], f32, name=f"Msb{g}", tag=f"Msb{g}")
                nc.scalar.copy(Msb[:], ps_M[g][:])
                nc.vector.tensor_reduce(out16[:, o:o + 1], Msb[:],
                                        axis=mybir.AxisListType.X, op=A.min)
                nc.vector.tensor_reduce(out16[:, o + 1:o + 2], Msb[:],
                                        axis=mybir.AxisListType.X, op=A.max)
                sc1 = tpool.tile([K, G], f32, tag="sc1")
                nc.vector.scalar_tensor_tensor(
                    sc1[:], in0=ew_l[g][:], scalar=1.0, in1=Msb[:],
                    op0=A.mult, op1=A.mult,
                    accum_out=out16[:, o + 5:o + 6])
                sc2 = tpool.tile([K, G], f32, tag="sc2")
                nc.vector.scalar_tensor_tensor(
                    sc2[:], in0=es_l[g][:], scalar=1.0, in1=Msb[:],
                    op0=A.mult, op1=A.mult,
                    accum_out=out16[:, o + 3:o + 4])

            nc.vector.memset(out16[:, 14:16], 0.0)
            nc.gpsimd.dma_start(out_ap[:], out16[:])

    nc.compile()
    return nc


def _get_program():
    global _COMPILED
    if _COMPILED is None:
        _COMPILED = _build()
    return _COMPILED


def _perm_k128(a):
    """[8192, n] -> [128, 64*n] with out[p, kt*n + j] = a[128*kt + p, j]."""
    n = a.shape[1]
    return np.ascontiguousarray(
        a.reshape(KT, 128, n).transpose(1, 0, 2).reshape(128, KT * n))


def kernel(beta, coherence_weight, epoch):
    from concourse.bass_utils import run_bass_kernel_spmd

    global LAST_RESULT
    beta = np.ascontiguousarray(np.asarray(beta, dtype=np.float32))
    W = np.asarray(coherence_weight, dtype=np.float32)
    epoch_i = int(np.asarray(epoch))

    nc = _get_program()

    betaTp = _perm_k128(np.ascontiguousarray(beta.T))
    ident = np.eye(K, dtype=np.float32)
    in_maps = []
    for c in range(N_CORES):
        sl = slice(c * CS, (c + 1) * CS)
        in_maps.append({
            "beta": beta,
            "betaTp": betaTp,
            "beta_s": np.ascontiguousarray(beta[:, sl]),
            "wp0": _perm_k128(W[:, c * CS:c * CS + G]),
            "wp1": _perm_k128(W[:, c * CS + G:(c + 1) * CS]),
            "ident": ident,
        })

    res = run_bass_kernel_spmd(nc, in_maps, core_ids=list(range(N_CORES)),
                               trace=TRACE)
    LAST_RESULT = res
    outs = np.stack([res.results[c]["out16"] for c in range(N_CORES)])  # [8,100,16]

    # ---- host combine (tiny: 8*100*16 floats -> scalar) ----
    o = outs.astype(np.float64)
    mn = np.minimum(o[:, :, 0], o[:, :, 6]).min(0)      # [100]
    mx = np.maximum(o[:, :, 1], o[:, :, 7]).max(0)
    T1 = (o[:, :, 2] + o[:, :, 8]).sum(0)
    T2 = (o[:, :, 3] + o[:, :, 9]).sum(0)
    P1 = (o[:, :, 4] + o[:, :, 10]).sum(0)
    P2 = (o[:, :, 5] + o[:, :, 11]).sum(0)
    R = o[0, :, 12]

    denom = mx - mn
    pos = (100.0 / R**2 * (mx * P1 - P2) / denom).sum()
    s_all = (100.0 / R**2 * (mx * T1 - T2) / denom).sum()
    neg = s_all - pos
    total = (pos * LAMBDA_D + neg * (1.0 - LAMBDA_D)) * 2.0
    lam_a = (epoch_i * (LAMBDA_A / WARMUP_EPOCHS)
             if epoch_i < WARMUP_EPOCHS else LAMBDA_A)
    return np.float32(lam_a * total)
